# revision 17
# baseline (speedup 1.0000x reference)
"""CNF ODE-function kernel for Trainium2 (8 NeuronCores, pure data parallel).

Computes, for B=262144 samples (z in R^8), two tiny tanh-MLPs:
  net1: dz/dt field (8->64->64->64->8), net2: growth g (8->64->64->64->1),
plus the exact trace of d(net1)/dz via a rank-8 meet-in-the-middle JVP:

  trace = sum_r a3[r] * u[r],   u = sum_{i,q} WU[(i,q),r] * F~[(i,q)]
  F~[(i,q)] = a2[q] * sum_k a1[k] * Wz[i,k]*W1_1[k,q]
  a_l = 1 - tanh^2(pre_l)   (layer derivative of net1)

All per-sample GEMMs run with features on partitions and batch on the free
dim; net1/net2 forward layers are fused block-diagonally into single
128-wide matmuls. Weights are tiny and replicated per core; the batch is
sharded 8 ways.
"""

import numpy as np
from contextlib import ExitStack

import concourse.bass as bass
import concourse.mybir as mybir
import concourse.tile as tile
from concourse import bacc
from concourse.bass import ts
from concourse.bass_utils import run_bass_kernel_spmd
from concourse.mybir import AluOpType as Op

AF = mybir.ActivationFunctionType
F32 = mybir.dt.float32
F32R = mybir.dt.float32r
B, D, H = 262144, 8, 64
NCORES = 8
BC = B // NCORES  # 32768 samples per core
FREE = 512        # batch elements per tile
NT = BC // FREE   # 64 tiles per core

_BUILD_CACHE = {}


def _build(bc: int):
    """Build the per-core Bass program for a batch shard of `bc` samples."""
    nt = bc // FREE
    nc = bacc.Bacc(None, target_bir_lowering=False)

    zt = nc.dram_tensor("zt", [D, bc], F32R, kind="ExternalInput")
    w0 = nc.dram_tensor("w0", [D, 128], F32R, kind="ExternalInput")
    b0 = nc.dram_tensor("b0", [128, 1], F32, kind="ExternalInput")
    w1 = nc.dram_tensor("w1", [128, 128], F32R, kind="ExternalInput")
    b1 = nc.dram_tensor("b1", [128, 1], F32, kind="ExternalInput")
    w2 = nc.dram_tensor("w2", [128, 128], F32R, kind="ExternalInput")
    b2 = nc.dram_tensor("b2", [128, 1], F32, kind="ExternalInput")
    wo = nc.dram_tensor("wo", [128, 9], F32R, kind="ExternalInput")
    bo = nc.dram_tensor("bo", [9, 1], F32, kind="ExternalInput")
    wf = nc.dram_tensor("wf", [128, 4 * H], F32R, kind="ExternalInput")
    cf = nc.dram_tensor("cf", [128, 4], F32, kind="ExternalInput")
    wu = nc.dram_tensor("wu", [8 * H, H], F32R, kind="ExternalInput")
    on = nc.dram_tensor("on", [128, 1], F32R, kind="ExternalInput")
    out = nc.dram_tensor("out", [10, bc], F32, kind="ExternalOutput")

    with tile.TileContext(nc) as tc, ExitStack() as ctx:
        cpool = ctx.enter_context(tc.tile_pool(name="const", bufs=1))
        w0_sb = cpool.tile([D, 128], F32R)
        nc.sync.dma_start(w0_sb[:], w0[:])
        b0_sb = cpool.tile([128, 1], F32)
        nc.sync.dma_start(b0_sb[:], b0[:])
        w1_sb = cpool.tile([128, 128], F32R)
        nc.sync.dma_start(w1_sb[:], w1[:])
        b1_sb = cpool.tile([128, 1], F32)
        nc.sync.dma_start(b1_sb[:], b1[:])
        w2_sb = cpool.tile([128, 128], F32R)
        nc.sync.dma_start(w2_sb[:], w2[:])
        b2_sb = cpool.tile([128, 1], F32)
        nc.sync.dma_start(b2_sb[:], b2[:])
        wo_sb = cpool.tile([128, 9], F32R)
        nc.sync.dma_start(wo_sb[:], wo[:])
        bo_sb = cpool.tile([9, 1], F32)
        nc.sync.dma_start(bo_sb[:], bo[:])
        wf_sb = cpool.tile([128, 4 * H], F32R)
        nc.sync.dma_start(wf_sb[:], wf[:])
        cf_sb = cpool.tile([128, 4], F32)
        nc.sync.dma_start(cf_sb[:], cf[:])
        wu_sb = cpool.tile([128, 4 * H], F32R)
        for j in range(4):
            nc.sync.dma_start(wu_sb[:, ts(j, H)], wu[ts(j, 128), :])
        on_sb = cpool.tile([128, 1], F32R)
        nc.sync.dma_start(on_sb[:], on[:])

        p_z = ctx.enter_context(tc.tile_pool(name="zin", bufs=4))
        p_psL = ctx.enter_context(tc.tile_pool(name="psL", bufs=2, space="PSUM"))
        p_psF = ctx.enter_context(tc.tile_pool(name="psF", bufs=4, space="PSUM"))
        p_psU = ctx.enter_context(tc.tile_pool(name="psU", bufs=1, space="PSUM"))
        p_psT = ctx.enter_context(tc.tile_pool(name="psT", bufs=1, space="PSUM"))
        p_h = ctx.enter_context(tc.tile_pool(name="h", bufs=3))
        p_sq = ctx.enter_context(tc.tile_pool(name="sq", bufs=2))
        p_a2 = ctx.enter_context(tc.tile_pool(name="a2", bufs=2))
        p_a3 = ctx.enter_context(tc.tile_pool(name="a3", bufs=2))
        p_Ft = ctx.enter_context(tc.tile_pool(name="Ft", bufs=2))
        p_u = ctx.enter_context(tc.tile_pool(name="u", bufs=2))
        p_st = ctx.enter_context(tc.tile_pool(name="st", bufs=3))
        p_tr = ctx.enter_context(tc.tile_pool(name="tr", bufs=3))

        for i in range(nt):
            zin = p_z.tile([D, FREE], F32R)
            nc.sync.dma_start(zin[:], zt[:, ts(i, FREE)])

            # forward, both nets fused on 128 partitions (net1: 0-63, net2: 64-127)
            psA = p_psL.tile([128, FREE], F32, tag="psL")
            nc.tensor.matmul(psA[:], w0_sb[:], zin[:], start=True, stop=True)
            h1 = p_h.tile([128, FREE], F32R, tag="h")
            nc.scalar.activation(h1[:], psA[:], AF.Tanh, bias=b0_sb[:])

            psB = p_psL.tile([128, FREE], F32, tag="psL")
            nc.tensor.matmul(psB[:], w1_sb[:], h1[:], start=True, stop=True)
            h2 = p_h.tile([128, FREE], F32R, tag="h")
            nc.scalar.activation(h2[:], psB[:], AF.Tanh, bias=b1_sb[:])

            psC = p_psL.tile([128, FREE], F32, tag="psL")
            nc.tensor.matmul(psC[:], w2_sb[:], h2[:], start=True, stop=True)
            h3 = p_h.tile([128, FREE], F32R, tag="h")
            nc.scalar.activation(h3[:], psC[:], AF.Tanh, bias=b2_sb[:])

            psD = p_psL.tile([128, FREE], F32, tag="psL")
            nc.tensor.matmul(psD[0:9, :], wo_sb[:], h3[:], start=True, stop=True)

            st = p_st.tile([9, FREE], F32)
            nc.scalar.activation(st[:], psD[0:9, :], AF.Identity, bias=bo_sb[:])

            # layer derivatives of net1: a_l = 1 - x_l^2
            # sq1 duplicated onto both partition halves for row-packed F-matmuls
            sq1d = p_sq.tile([128, FREE], F32R, tag="sq1")
            nc.scalar.activation(sq1d[0:H, :], h1[0:H, :], AF.Square)
            nc.gpsimd.tensor_scalar(sq1d[H:128, :], sq1d[0:H, :], 0.0, None, Op.add)
            sq2 = p_sq.tile([H, FREE], F32, tag="sq2")
            nc.vector.scalar_tensor_tensor(
                sq2[:], h2[0:H, :], 0.0, h2[0:H, :], Op.add, Op.mult
            )
            a2rep = p_a2.tile([128, FREE], F32)
            nc.gpsimd.tensor_scalar(a2rep[0:H, :], sq2[:], -1.0, 1.0, Op.mult, Op.add)
            nc.gpsimd.tensor_scalar(a2rep[H:128, :], sq2[:], -1.0, 1.0, Op.mult, Op.add)
            sq3 = p_sq.tile([H, FREE], F32, tag="sq3")
            nc.gpsimd.tensor_tensor(sq3[:], h3[0:H, :], h3[0:H, :], Op.mult)
            a3d = p_a3.tile([H, FREE], F32)
            nc.gpsimd.tensor_scalar(a3d[:], sq3[:], -1.0, 1.0, Op.mult, Op.add)

            # JVP forward half: F~ = (-WF^T sq1 + cf) * a2rep, 2 tangents per block.
            # Row-packed: blocks 0,1 run on PE rows 0-63, blocks 2,3 on rows 64-127
            # concurrently (disjoint 32x32 PE tiles).
            Ft = p_Ft.tile([128, 4 * FREE], F32R)
            psF = [None] * 4
            for b in range(2):
                for blk, rows in ((b, slice(0, H)), (2 + b, slice(H, 128))):
                    psF[blk] = p_psF.tile([128, FREE], F32, tag="psF", name=f"psF{blk}")
                    nc.tensor.matmul(
                        psF[blk][:], wf_sb[rows, ts(b, 128)], sq1d[rows, :],
                        start=True, stop=True,
                    )
            for blk in range(4):
                nc.vector.scalar_tensor_tensor(
                    Ft[:, ts(blk, FREE)], psF[blk][:], cf_sb[:, blk : blk + 1],
                    a2rep[:], Op.add, Op.mult,
                )

            # JVP backward half folded into constants: u = WU^T F~.
            # Col-packed: blocks 0,1 accumulate into psU rows 0-63, blocks 2,3
            # into rows 64-127 (disjoint PE column groups run concurrently).
            psU = p_psU.tile([H, FREE], F32)
            for j in range(4):
                nc.tensor.matmul(
                    psU[:], wu_sb[:, ts(j, H)], Ft[:, ts(j, FREE)],
                    start=(j == 0), stop=(j == 3),
                )
            ua3 = p_u.tile([H, FREE], F32R)
            nc.vector.scalar_tensor_tensor(
                ua3[:], psU[:], 0.0, a3d[:], Op.add, Op.mult
            )

            # trace = ones^T (u * a3)  (partition reduction via PE)
            psT = p_psT.tile([1, FREE], F32)
            nc.tensor.matmul(psT[:], on_sb[0:H, :], ua3[:], start=True, stop=True)
            tr = p_tr.tile([1, FREE], F32)
            nc.scalar.activation(tr[:], psT[:], AF.Copy)

            nc.sync.dma_start(out[0:9, ts(i, FREE)], st[:])
            nc.sync.dma_start(out[9:10, ts(i, FREE)], tr[:])

    nc.finalize()
    return nc


def _constants(t, w1_0, b1_0, w1_1, b1_1, w1_2, b1_2, w1_o, b1_o,
               w2_0, b2_0, w2_1, b2_1, w2_2, b2_2, w2_o, b2_o):
    f = np.float32
    t = f(t)
    w0cat = np.concatenate([w1_0[1:], w2_0[1:]], axis=1).astype(f)      # (8,128)
    b0cat = np.concatenate([t * w1_0[0] + b1_0, t * w2_0[0] + b2_0])[:, None].astype(f)
    w1cat = np.zeros((128, 128), f)
    w1cat[:H, :H] = w1_1
    w1cat[H:, H:] = w2_1
    b1cat = np.concatenate([b1_1, b2_1])[:, None].astype(f)
    w2cat = np.zeros((128, 128), f)
    w2cat[:H, :H] = w1_2
    w2cat[H:, H:] = w2_2
    b2cat = np.concatenate([b1_2, b2_2])[:, None].astype(f)
    wocat = np.zeros((128, 9), f)
    wocat[:H, :D] = w1_o
    wocat[H:, D:] = w2_o
    bocat = np.concatenate([b1_o, b2_o])[:, None].astype(f)
    wz = w1_0[1:]                                                        # (8,64)
    what = np.einsum("ik,kq->kiq", wz, w1_1).reshape(H, 8 * H).astype(f)
    # row-packed F weights: blocks 0,1 on partitions 0-63, blocks 2,3 on 64-127
    wf = np.zeros((128, 256), f)
    wf[:H, :] = -what[:, 0:256]
    wf[H:, :] = -what[:, 256:512]
    cf = np.ascontiguousarray(what.sum(0).reshape(4, 128).T)             # (128,4)
    wu = np.einsum("qr,ri->iqr", w1_2, w1_o).reshape(8 * H, H).astype(f)
    on = np.ones((128, 1), f)
    return dict(w0=w0cat, b0=b0cat, w1=w1cat, b1=b1cat, w2=w2cat, b2=b2cat,
                wo=wocat, bo=bocat, wf=wf, cf=cf,
                wu=np.ascontiguousarray(wu), on=on)


def kernel(t, z, logp_z,
           w1_0, b1_0, w1_1, b1_1, w1_2, b1_2, w1_o, b1_o,
           w2_0, b2_0, w2_1, b2_1, w2_2, b2_2, w2_o, b2_o,
           _trace=False):
    consts = _constants(t, w1_0, b1_0, w1_1, b1_1, w1_2, b1_2, w1_o, b1_o,
                        w2_0, b2_0, w2_1, b2_1, w2_2, b2_2, w2_o, b2_o)
    zt = np.ascontiguousarray(np.asarray(z, np.float32).T)               # (8, B)

    bc = zt.shape[1] // NCORES
    if bc not in _BUILD_CACHE:
        _BUILD_CACHE[bc] = _build(bc)
    nc = _BUILD_CACHE[bc]

    in_maps = []
    for c in range(NCORES):
        m = dict(consts)
        m["zt"] = np.ascontiguousarray(zt[:, c * bc : (c + 1) * bc])
        in_maps.append(m)

    res = run_bass_kernel_spmd(nc, in_maps, list(range(NCORES)), trace=_trace)
    outs = np.concatenate([r["out"] for r in res.results], axis=1)       # (10, B)

    dz_dt = np.ascontiguousarray(outs[0:D].T)
    g = np.ascontiguousarray(outs[D : D + 1].T)
    dlogp = np.ascontiguousarray((outs[D] - outs[D + 1])[:, None])
    if _trace:
        return (dz_dt, dlogp, g), res
    return (dz_dt, dlogp, g)


# revision 18
# speedup vs baseline: 2.2311x; 2.2311x over previous
"""CNF ODE-function kernel for Trainium2 (8 NeuronCores, pure data parallel).

Computes, for B=262144 samples (z in R^8), two tiny tanh-MLPs:
  net1: dz/dt field (8->64->64->64->8), net2: growth g (8->64->64->64->1),
plus the exact trace of d(net1)/dz via a rank-8 meet-in-the-middle JVP:

  trace = sum_r a3[r] * u[r],   u = sum_{i,q} WU[(i,q),r] * F~[(i,q)]
  F~[(i,q)] = a2[q] * sum_k a1[k] * Wz[i,k]*W1_1[k,q]
  a_l = 1 - tanh^2(pre_l)   (layer derivative of net1)

All per-sample GEMMs run with features on partitions and batch on the free
dim; net1/net2 forward layers are fused block-diagonally into single
128-wide matmuls. Weights are tiny and replicated per core; the batch is
sharded 8 ways.
"""

import numpy as np
from contextlib import ExitStack

import concourse.bass as bass
import concourse.mybir as mybir
import concourse.tile as tile
from concourse import bacc
from concourse.bass import ts
from concourse.bass_utils import run_bass_kernel_spmd
from concourse.mybir import AluOpType as Op

AF = mybir.ActivationFunctionType
F32 = mybir.dt.float32
F32R = mybir.dt.float32r
B, D, H = 262144, 8, 64
NCORES = 8
BC = B // NCORES  # 32768 samples per core
FREE = 512        # batch elements per tile
NT = BC // FREE   # 64 tiles per core

_BUILD_CACHE = {}


def _build(bc: int):
    """Build the per-core Bass program for a batch shard of `bc` samples."""
    nt = bc // FREE
    nc = bacc.Bacc(None, target_bir_lowering=False)

    zt = nc.dram_tensor("zt", [D, bc], F32R, kind="ExternalInput")
    w0 = nc.dram_tensor("w0", [D, 128], F32R, kind="ExternalInput")
    b0 = nc.dram_tensor("b0", [128, 1], F32, kind="ExternalInput")
    w1 = nc.dram_tensor("w1", [128, 128], F32R, kind="ExternalInput")
    b1 = nc.dram_tensor("b1", [128, 1], F32, kind="ExternalInput")
    w2 = nc.dram_tensor("w2", [128, 128], F32R, kind="ExternalInput")
    b2 = nc.dram_tensor("b2", [128, 1], F32, kind="ExternalInput")
    wo = nc.dram_tensor("wo", [128, 9], F32R, kind="ExternalInput")
    bo = nc.dram_tensor("bo", [9, 1], F32, kind="ExternalInput")
    wf = nc.dram_tensor("wf", [128, 4 * H], F32R, kind="ExternalInput")
    cf = nc.dram_tensor("cf", [128, 4], F32, kind="ExternalInput")
    wu = nc.dram_tensor("wu", [8 * H, H], F32R, kind="ExternalInput")
    on = nc.dram_tensor("on", [128, 1], F32R, kind="ExternalInput")
    out = nc.dram_tensor("out", [10, bc], F32, kind="ExternalOutput")

    with tile.TileContext(nc) as tc, ExitStack() as ctx:
        cpool = ctx.enter_context(tc.tile_pool(name="const", bufs=1))
        w0_sb = cpool.tile([D, 128], F32R)
        nc.sync.dma_start(w0_sb[:], w0[:])
        b0_sb = cpool.tile([128, 1], F32)
        nc.sync.dma_start(b0_sb[:], b0[:])
        w1_sb = cpool.tile([128, 128], F32R)
        nc.sync.dma_start(w1_sb[:], w1[:])
        b1_sb = cpool.tile([128, 1], F32)
        nc.sync.dma_start(b1_sb[:], b1[:])
        w2_sb = cpool.tile([128, 128], F32R)
        nc.sync.dma_start(w2_sb[:], w2[:])
        b2_sb = cpool.tile([128, 1], F32)
        nc.sync.dma_start(b2_sb[:], b2[:])
        wo_sb = cpool.tile([128, 9], F32R)
        nc.sync.dma_start(wo_sb[:], wo[:])
        bo_sb = cpool.tile([9, 1], F32)
        nc.sync.dma_start(bo_sb[:], bo[:])
        wf_sb = cpool.tile([128, 4 * H], F32R)
        nc.sync.dma_start(wf_sb[:], wf[:])
        cf_sb = cpool.tile([128, 4], F32)
        nc.sync.dma_start(cf_sb[:], cf[:])
        wu_sb = cpool.tile([128, 4 * H], F32R)
        for j in range(4):
            nc.sync.dma_start(wu_sb[:, ts(j, H)], wu[ts(j, 128), :])
        on_sb = cpool.tile([128, 1], F32R)
        nc.sync.dma_start(on_sb[:], on[:])

        p_z = ctx.enter_context(tc.tile_pool(name="zin", bufs=4))
        p_psL = ctx.enter_context(tc.tile_pool(name="psL", bufs=2, space="PSUM"))
        p_psF = ctx.enter_context(tc.tile_pool(name="psF", bufs=4, space="PSUM"))
        p_psU = ctx.enter_context(tc.tile_pool(name="psU", bufs=1, space="PSUM"))
        p_psT = ctx.enter_context(tc.tile_pool(name="psT", bufs=1, space="PSUM"))
        p_h = ctx.enter_context(tc.tile_pool(name="h", bufs=3))
        p_sq = ctx.enter_context(tc.tile_pool(name="sq", bufs=2))
        p_a2 = ctx.enter_context(tc.tile_pool(name="a2", bufs=2))
        p_a3 = ctx.enter_context(tc.tile_pool(name="a3", bufs=2))
        p_Ft = ctx.enter_context(tc.tile_pool(name="Ft", bufs=2))
        p_u = ctx.enter_context(tc.tile_pool(name="u", bufs=2))
        p_st = ctx.enter_context(tc.tile_pool(name="st", bufs=3))
        p_tr = ctx.enter_context(tc.tile_pool(name="tr", bufs=3))

        for i in range(nt):
            zin = p_z.tile([D, FREE], F32R)
            nc.sync.dma_start(zin[:], zt[:, ts(i, FREE)])

            # forward, both nets fused on 128 partitions (net1: 0-63, net2: 64-127)
            psA = p_psL.tile([128, FREE], F32, tag="psL")
            nc.tensor.matmul(psA[:], w0_sb[:], zin[:], start=True, stop=True)
            h1 = p_h.tile([128, FREE], F32R, tag="h")
            nc.scalar.activation(h1[:], psA[:], AF.Tanh, bias=b0_sb[:])

            psB = p_psL.tile([128, FREE], F32, tag="psL")
            nc.tensor.matmul(psB[:], w1_sb[:], h1[:], start=True, stop=True)
            h2 = p_h.tile([128, FREE], F32R, tag="h")
            nc.scalar.activation(h2[:], psB[:], AF.Tanh, bias=b1_sb[:])

            psC = p_psL.tile([128, FREE], F32, tag="psL")
            nc.tensor.matmul(psC[:], w2_sb[:], h2[:], start=True, stop=True)
            h3 = p_h.tile([128, FREE], F32R, tag="h")
            nc.scalar.activation(h3[:], psC[:], AF.Tanh, bias=b2_sb[:])

            psD = p_psL.tile([128, FREE], F32, tag="psL")
            nc.tensor.matmul(psD[0:9, :], wo_sb[:], h3[:], start=True, stop=True)

            st = p_st.tile([9, FREE], F32)
            nc.scalar.activation(st[:], psD[0:9, :], AF.Identity, bias=bo_sb[:])

            # layer derivatives of net1: a_l = 1 - x_l^2
            # sq1 duplicated onto both partition halves for row-packed F-matmuls
            sq1d = p_sq.tile([128, FREE], F32R, tag="sq1")
            nc.scalar.activation(sq1d[0:H, :], h1[0:H, :], AF.Square)
            nc.scalar.activation(sq1d[H:128, :], h1[0:H, :], AF.Square)
            sq2 = p_sq.tile([H, FREE], F32, tag="sq2")
            nc.vector.scalar_tensor_tensor(
                sq2[:], h2[0:H, :], 0.0, h2[0:H, :], Op.add, Op.mult
            )
            a2rep = p_a2.tile([128, FREE], F32)
            nc.gpsimd.tensor_scalar(a2rep[0:H, :], sq2[:], -1.0, 1.0, Op.mult, Op.add)
            nc.gpsimd.tensor_scalar(a2rep[H:128, :], sq2[:], -1.0, 1.0, Op.mult, Op.add)
            sq3 = p_sq.tile([H, FREE], F32, tag="sq3")
            nc.gpsimd.tensor_tensor(sq3[:], h3[0:H, :], h3[0:H, :], Op.mult)
            a3d = p_a3.tile([H, FREE], F32)
            nc.gpsimd.tensor_scalar(a3d[:], sq3[:], -1.0, 1.0, Op.mult, Op.add)

            # JVP forward half: F~ = (-WF^T sq1 + cf) * a2rep, 2 tangents per block.
            # Row-packed: blocks 0,1 run on PE rows 0-63, blocks 2,3 on rows 64-127
            # concurrently (disjoint 32x32 PE tiles).
            Ft = p_Ft.tile([128, 4 * FREE], F32R)
            psF = [None] * 4
            for b in range(2):
                for blk, rows in ((b, slice(0, H)), (2 + b, slice(H, 128))):
                    psF[blk] = p_psF.tile([128, FREE], F32, tag="psF", name=f"psF{blk}")
                    nc.tensor.matmul(
                        psF[blk][:], wf_sb[rows, ts(b, 128)], sq1d[rows, :],
                        start=True, stop=True,
                    )
            for blk in range(4):
                nc.vector.scalar_tensor_tensor(
                    Ft[:, ts(blk, FREE)], psF[blk][:], cf_sb[:, blk : blk + 1],
                    a2rep[:], Op.add, Op.mult,
                )

            # JVP backward half folded into constants: u = WU^T F~.
            # Col-packed: blocks 0,1 accumulate into psU rows 0-63, blocks 2,3
            # into rows 64-127 (disjoint PE column groups run concurrently).
            psU = p_psU.tile([H, FREE], F32)
            for j in range(4):
                nc.tensor.matmul(
                    psU[:], wu_sb[:, ts(j, H)], Ft[:, ts(j, FREE)],
                    start=(j == 0), stop=(j == 3),
                )
            ua3 = p_u.tile([H, FREE], F32R)
            nc.vector.scalar_tensor_tensor(
                ua3[:], psU[:], 0.0, a3d[:], Op.add, Op.mult
            )

            # trace = ones^T (u * a3)  (partition reduction via PE)
            psT = p_psT.tile([1, FREE], F32)
            nc.tensor.matmul(psT[:], on_sb[0:H, :], ua3[:], start=True, stop=True)
            tr = p_tr.tile([1, FREE], F32)
            nc.scalar.activation(tr[:], psT[:], AF.Copy)

            nc.sync.dma_start(out[0:9, ts(i, FREE)], st[:])
            nc.sync.dma_start(out[9:10, ts(i, FREE)], tr[:])

    nc.finalize()
    return nc


def _constants(t, w1_0, b1_0, w1_1, b1_1, w1_2, b1_2, w1_o, b1_o,
               w2_0, b2_0, w2_1, b2_1, w2_2, b2_2, w2_o, b2_o):
    f = np.float32
    t = f(t)
    w0cat = np.concatenate([w1_0[1:], w2_0[1:]], axis=1).astype(f)      # (8,128)
    b0cat = np.concatenate([t * w1_0[0] + b1_0, t * w2_0[0] + b2_0])[:, None].astype(f)
    w1cat = np.zeros((128, 128), f)
    w1cat[:H, :H] = w1_1
    w1cat[H:, H:] = w2_1
    b1cat = np.concatenate([b1_1, b2_1])[:, None].astype(f)
    w2cat = np.zeros((128, 128), f)
    w2cat[:H, :H] = w1_2
    w2cat[H:, H:] = w2_2
    b2cat = np.concatenate([b1_2, b2_2])[:, None].astype(f)
    wocat = np.zeros((128, 9), f)
    wocat[:H, :D] = w1_o
    wocat[H:, D:] = w2_o
    bocat = np.concatenate([b1_o, b2_o])[:, None].astype(f)
    wz = w1_0[1:]                                                        # (8,64)
    what = np.einsum("ik,kq->kiq", wz, w1_1).reshape(H, 8 * H).astype(f)
    # row-packed F weights: blocks 0,1 on partitions 0-63, blocks 2,3 on 64-127
    wf = np.zeros((128, 256), f)
    wf[:H, :] = -what[:, 0:256]
    wf[H:, :] = -what[:, 256:512]
    cf = np.ascontiguousarray(what.sum(0).reshape(4, 128).T)             # (128,4)
    wu = np.einsum("qr,ri->iqr", w1_2, w1_o).reshape(8 * H, H).astype(f)
    on = np.ones((128, 1), f)
    return dict(w0=w0cat, b0=b0cat, w1=w1cat, b1=b1cat, w2=w2cat, b2=b2cat,
                wo=wocat, bo=bocat, wf=wf, cf=cf,
                wu=np.ascontiguousarray(wu), on=on)


def kernel(t, z, logp_z,
           w1_0, b1_0, w1_1, b1_1, w1_2, b1_2, w1_o, b1_o,
           w2_0, b2_0, w2_1, b2_1, w2_2, b2_2, w2_o, b2_o,
           _trace=False):
    consts = _constants(t, w1_0, b1_0, w1_1, b1_1, w1_2, b1_2, w1_o, b1_o,
                        w2_0, b2_0, w2_1, b2_1, w2_2, b2_2, w2_o, b2_o)
    zt = np.ascontiguousarray(np.asarray(z, np.float32).T)               # (8, B)

    bc = zt.shape[1] // NCORES
    if bc not in _BUILD_CACHE:
        _BUILD_CACHE[bc] = _build(bc)
    nc = _BUILD_CACHE[bc]

    in_maps = []
    for c in range(NCORES):
        m = dict(consts)
        m["zt"] = np.ascontiguousarray(zt[:, c * bc : (c + 1) * bc])
        in_maps.append(m)

    res = run_bass_kernel_spmd(nc, in_maps, list(range(NCORES)), trace=_trace)
    outs = np.concatenate([r["out"] for r in res.results], axis=1)       # (10, B)

    dz_dt = np.ascontiguousarray(outs[0:D].T)
    g = np.ascontiguousarray(outs[D : D + 1].T)
    dlogp = np.ascontiguousarray((outs[D] - outs[D + 1])[:, None])
    if _trace:
        return (dz_dt, dlogp, g), res
    return (dz_dt, dlogp, g)


# revision 19
# speedup vs baseline: 22.1366x; 9.9217x over previous
"""CNF ODE-function kernel for Trainium2 (8 NeuronCores, pure data parallel).

Computes, for B=262144 samples (z in R^8), two tiny tanh-MLPs:
  net1: dz/dt field (8->64->64->64->8), net2: growth g (8->64->64->64->1),
plus the exact trace of d(net1)/dz via a rank-8 meet-in-the-middle JVP:

  trace = sum_r a3[r] * u[r],   u = sum_{i,q} WU[(i,q),r] * F~[(i,q)]
  F~[(i,q)] = a2[q] * sum_k a1[k] * Wz[i,k]*W1_1[k,q]
  a_l = 1 - tanh^2(pre_l)   (layer derivative of net1)

All per-sample GEMMs run with features on partitions and batch on the free
dim; net1/net2 forward layers are fused block-diagonally into single
128-wide matmuls. Weights are tiny and replicated per core; the batch is
sharded 8 ways.
"""

import numpy as np
from contextlib import ExitStack

import concourse.bass as bass
import concourse.mybir as mybir
import concourse.tile as tile
from concourse import bacc
from concourse.bass import ts
from concourse.bass_utils import run_bass_kernel_spmd
from concourse.mybir import AluOpType as Op

AF = mybir.ActivationFunctionType
F32 = mybir.dt.float32
F32R = mybir.dt.float32r
B, D, H = 262144, 8, 64
NCORES = 8
BC = B // NCORES  # 32768 samples per core
FREE = 512        # batch elements per tile
NT = BC // FREE   # 64 tiles per core

_BUILD_CACHE = {}


def _build(bc: int):
    """Build the per-core Bass program for a batch shard of `bc` samples."""
    nt = bc // FREE
    nc = bacc.Bacc(None, target_bir_lowering=False)

    zt = nc.dram_tensor("zt", [D, bc], F32R, kind="ExternalInput")
    w0 = nc.dram_tensor("w0", [D, 128], F32R, kind="ExternalInput")
    b0 = nc.dram_tensor("b0", [128, 1], F32, kind="ExternalInput")
    w1 = nc.dram_tensor("w1", [128, 128], F32R, kind="ExternalInput")
    b1 = nc.dram_tensor("b1", [128, 1], F32, kind="ExternalInput")
    w2 = nc.dram_tensor("w2", [128, 128], F32R, kind="ExternalInput")
    b2 = nc.dram_tensor("b2", [128, 1], F32, kind="ExternalInput")
    wo = nc.dram_tensor("wo", [128, 9], F32R, kind="ExternalInput")
    bo = nc.dram_tensor("bo", [9, 1], F32, kind="ExternalInput")
    wf = nc.dram_tensor("wf", [128, 4 * H], F32R, kind="ExternalInput")
    cf = nc.dram_tensor("cf", [128, 4], F32, kind="ExternalInput")
    wu = nc.dram_tensor("wu", [8 * H, H], F32R, kind="ExternalInput")
    on = nc.dram_tensor("on", [128, 1], F32R, kind="ExternalInput")
    out = nc.dram_tensor("out", [10, bc], F32, kind="ExternalOutput")

    with tile.TileContext(nc) as tc, ExitStack() as ctx:
        cpool = ctx.enter_context(tc.tile_pool(name="const", bufs=1))
        w0_sb = cpool.tile([D, 128], F32R)
        nc.sync.dma_start(w0_sb[:], w0[:])
        b0_sb = cpool.tile([128, 1], F32)
        nc.sync.dma_start(b0_sb[:], b0[:])
        w1_sb = cpool.tile([128, 128], F32R)
        nc.sync.dma_start(w1_sb[:], w1[:])
        b1_sb = cpool.tile([128, 1], F32)
        nc.sync.dma_start(b1_sb[:], b1[:])
        w2_sb = cpool.tile([128, 128], F32R)
        nc.sync.dma_start(w2_sb[:], w2[:])
        b2_sb = cpool.tile([128, 1], F32)
        nc.sync.dma_start(b2_sb[:], b2[:])
        wo_sb = cpool.tile([128, 9], F32R)
        nc.sync.dma_start(wo_sb[:], wo[:])
        bo_sb = cpool.tile([9, 1], F32)
        nc.sync.dma_start(bo_sb[:], bo[:])
        wf_sb = cpool.tile([128, 4 * H], F32R)
        nc.sync.dma_start(wf_sb[:], wf[:])
        cf_sb = cpool.tile([128, 4], F32)
        nc.sync.dma_start(cf_sb[:], cf[:])
        wu_sb = cpool.tile([128, 4 * H], F32R)
        for j in range(4):
            nc.sync.dma_start(wu_sb[:, ts(j, H)], wu[ts(j, 128), :])
        on_sb = cpool.tile([128, 1], F32R)
        nc.sync.dma_start(on_sb[:], on[:])

        p_z = ctx.enter_context(tc.tile_pool(name="zin", bufs=4))
        p_psL = ctx.enter_context(tc.tile_pool(name="psL", bufs=2, space="PSUM"))
        p_psF = ctx.enter_context(tc.tile_pool(name="psF", bufs=4, space="PSUM"))
        p_psU = ctx.enter_context(tc.tile_pool(name="psU", bufs=1, space="PSUM"))
        p_psT = ctx.enter_context(tc.tile_pool(name="psT", bufs=1, space="PSUM"))
        p_h = ctx.enter_context(tc.tile_pool(name="h", bufs=3))
        p_sq = ctx.enter_context(tc.tile_pool(name="sq", bufs=2))
        p_a2 = ctx.enter_context(tc.tile_pool(name="a2", bufs=2))
        p_a3 = ctx.enter_context(tc.tile_pool(name="a3", bufs=2))
        p_Ft = ctx.enter_context(tc.tile_pool(name="Ft", bufs=2))
        p_u = ctx.enter_context(tc.tile_pool(name="u", bufs=2))
        p_st = ctx.enter_context(tc.tile_pool(name="st", bufs=3))
        p_tr = ctx.enter_context(tc.tile_pool(name="tr", bufs=3))

        for i in range(nt):
            zin = p_z.tile([D, FREE], F32R)
            nc.sync.dma_start(zin[:], zt[:, ts(i, FREE)])

            # forward, both nets fused on 128 partitions (net1: 0-63, net2: 64-127)
            psA = p_psL.tile([128, FREE], F32, tag="psL")
            nc.tensor.matmul(psA[:], w0_sb[:], zin[:], start=True, stop=True)
            h1 = p_h.tile([128, FREE], F32R, tag="h")
            nc.scalar.activation(h1[:], psA[:], AF.Tanh, bias=b0_sb[:])

            psB = p_psL.tile([128, FREE], F32, tag="psL")
            nc.tensor.matmul(psB[:], w1_sb[:], h1[:], start=True, stop=True)
            h2 = p_h.tile([128, FREE], F32R, tag="h")
            nc.scalar.activation(h2[:], psB[:], AF.Tanh, bias=b1_sb[:])

            psC = p_psL.tile([128, FREE], F32, tag="psL")
            nc.tensor.matmul(psC[:], w2_sb[:], h2[:], start=True, stop=True)
            h3 = p_h.tile([128, FREE], F32R, tag="h")
            nc.scalar.activation(h3[:], psC[:], AF.Tanh, bias=b2_sb[:])

            psD = p_psL.tile([128, FREE], F32, tag="psL")
            nc.tensor.matmul(psD[0:9, :], wo_sb[:], h3[:], start=True, stop=True)

            st = p_st.tile([9, FREE], F32)
            nc.scalar.activation(st[:], psD[0:9, :], AF.Identity, bias=bo_sb[:])

            # layer derivatives of net1: a_l = 1 - x_l^2
            # sq1 duplicated onto both partition halves for row-packed F-matmuls
            sq1d = p_sq.tile([128, FREE], F32R, tag="sq1")
            nc.scalar.activation(sq1d[0:H, :], h1[0:H, :], AF.Square)
            nc.scalar.activation(sq1d[H:128, :], h1[0:H, :], AF.Square)
            sq2 = p_sq.tile([H, FREE], F32, tag="sq2")
            nc.vector.scalar_tensor_tensor(
                sq2[:], h2[0:H, :], 0.0, h2[0:H, :], Op.add, Op.mult
            )
            a2rep = p_a2.tile([128, FREE], F32)
            nc.gpsimd.tensor_scalar(a2rep[0:H, :], sq2[:], -1.0, 1.0, Op.mult, Op.add)
            nc.gpsimd.tensor_scalar(a2rep[H:128, :], sq2[:], -1.0, 1.0, Op.mult, Op.add)
            sq3 = p_sq.tile([H, FREE], F32, tag="sq3")
            nc.gpsimd.tensor_tensor(sq3[:], h3[0:H, :], h3[0:H, :], Op.mult)
            a3d = p_a3.tile([H, FREE], F32)
            nc.gpsimd.tensor_scalar(a3d[:], sq3[:], -1.0, 1.0, Op.mult, Op.add)

            # JVP forward half: F~ = (-WF^T sq1 + cf) * a2rep, 2 tangents per block.
            # Row-packed: blocks 0,1 run on PE rows 0-63, blocks 2,3 on rows 64-127
            # concurrently (disjoint 32x32 PE tiles).
            Ft = p_Ft.tile([128, 4 * FREE], F32R)
            psF = [None] * 4
            for b in range(2):
                for blk, rows in ((b, slice(0, H)), (2 + b, slice(H, 128))):
                    psF[blk] = p_psF.tile([128, FREE], F32, tag="psF", name=f"psF{blk}")
                with tc.tile_critical():
                    for blk, rows in ((b, slice(0, H)), (2 + b, slice(H, 128))):
                        nc.tensor.matmul(
                            psF[blk][:], wf_sb[rows, ts(b, 128)], sq1d[rows, :],
                            start=True, stop=True,
                        )
            for blk in range(4):
                nc.vector.scalar_tensor_tensor(
                    Ft[:, ts(blk, FREE)], psF[blk][:], cf_sb[:, blk : blk + 1],
                    a2rep[:], Op.add, Op.mult,
                )

            # JVP backward half folded into constants: u = WU^T F~.
            # Col-packed: blocks 0,1 accumulate into psU rows 0-63, blocks 2,3
            # into rows 64-127 (disjoint PE column groups run concurrently).
            psU = p_psU.tile([H, FREE], F32)
            for j in range(4):
                nc.tensor.matmul(
                    psU[:], wu_sb[:, ts(j, H)], Ft[:, ts(j, FREE)],
                    start=(j == 0), stop=(j == 3),
                )
            ua3 = p_u.tile([H, FREE], F32R)
            nc.vector.scalar_tensor_tensor(
                ua3[:], psU[:], 0.0, a3d[:], Op.add, Op.mult
            )

            # trace = ones^T (u * a3)  (partition reduction via PE)
            psT = p_psT.tile([1, FREE], F32)
            nc.tensor.matmul(psT[:], on_sb[0:H, :], ua3[:], start=True, stop=True)
            tr = p_tr.tile([1, FREE], F32)
            nc.scalar.activation(tr[:], psT[:], AF.Copy)

            nc.sync.dma_start(out[0:9, ts(i, FREE)], st[:])
            nc.sync.dma_start(out[9:10, ts(i, FREE)], tr[:])

    nc.finalize()
    return nc


def _constants(t, w1_0, b1_0, w1_1, b1_1, w1_2, b1_2, w1_o, b1_o,
               w2_0, b2_0, w2_1, b2_1, w2_2, b2_2, w2_o, b2_o):
    f = np.float32
    t = f(t)
    w0cat = np.concatenate([w1_0[1:], w2_0[1:]], axis=1).astype(f)      # (8,128)
    b0cat = np.concatenate([t * w1_0[0] + b1_0, t * w2_0[0] + b2_0])[:, None].astype(f)
    w1cat = np.zeros((128, 128), f)
    w1cat[:H, :H] = w1_1
    w1cat[H:, H:] = w2_1
    b1cat = np.concatenate([b1_1, b2_1])[:, None].astype(f)
    w2cat = np.zeros((128, 128), f)
    w2cat[:H, :H] = w1_2
    w2cat[H:, H:] = w2_2
    b2cat = np.concatenate([b1_2, b2_2])[:, None].astype(f)
    wocat = np.zeros((128, 9), f)
    wocat[:H, :D] = w1_o
    wocat[H:, D:] = w2_o
    bocat = np.concatenate([b1_o, b2_o])[:, None].astype(f)
    wz = w1_0[1:]                                                        # (8,64)
    what = np.einsum("ik,kq->kiq", wz, w1_1).reshape(H, 8 * H).astype(f)
    # row-packed F weights: blocks 0,1 on partitions 0-63, blocks 2,3 on 64-127
    wf = np.zeros((128, 256), f)
    wf[:H, :] = -what[:, 0:256]
    wf[H:, :] = -what[:, 256:512]
    cf = np.ascontiguousarray(what.sum(0).reshape(4, 128).T)             # (128,4)
    wu = np.einsum("qr,ri->iqr", w1_2, w1_o).reshape(8 * H, H).astype(f)
    on = np.ones((128, 1), f)
    return dict(w0=w0cat, b0=b0cat, w1=w1cat, b1=b1cat, w2=w2cat, b2=b2cat,
                wo=wocat, bo=bocat, wf=wf, cf=cf,
                wu=np.ascontiguousarray(wu), on=on)


def kernel(t, z, logp_z,
           w1_0, b1_0, w1_1, b1_1, w1_2, b1_2, w1_o, b1_o,
           w2_0, b2_0, w2_1, b2_1, w2_2, b2_2, w2_o, b2_o,
           _trace=False):
    consts = _constants(t, w1_0, b1_0, w1_1, b1_1, w1_2, b1_2, w1_o, b1_o,
                        w2_0, b2_0, w2_1, b2_1, w2_2, b2_2, w2_o, b2_o)
    zt = np.ascontiguousarray(np.asarray(z, np.float32).T)               # (8, B)

    bc = zt.shape[1] // NCORES
    if bc not in _BUILD_CACHE:
        _BUILD_CACHE[bc] = _build(bc)
    nc = _BUILD_CACHE[bc]

    in_maps = []
    for c in range(NCORES):
        m = dict(consts)
        m["zt"] = np.ascontiguousarray(zt[:, c * bc : (c + 1) * bc])
        in_maps.append(m)

    res = run_bass_kernel_spmd(nc, in_maps, list(range(NCORES)), trace=_trace)
    outs = np.concatenate([r["out"] for r in res.results], axis=1)       # (10, B)

    dz_dt = np.ascontiguousarray(outs[0:D].T)
    g = np.ascontiguousarray(outs[D : D + 1].T)
    dlogp = np.ascontiguousarray((outs[D] - outs[D + 1])[:, None])
    if _trace:
        return (dz_dt, dlogp, g), res
    return (dz_dt, dlogp, g)
